# revision 1
# baseline (speedup 1.0000x reference)
"""Trainium2 Bass kernel for nn_CNN_Encoder_33328946217622.

Reference computation (per batch image):
  x: [144, 64, 64]
  x1 = relu(BN1(conv3x3(x, w1) + b1))     (96 ch)
  x2 = relu(BN2(conv5x5(x, w2) + b2))
  x3 = relu(BN3(conv7x7(x, w3) + b3))
  add = x1+x2+x3 ; avg = add/3 ; mx = max(x1,x2,x3)
  out = conv3x3_depthwise_shared([add, avg, mx], wf)   (3->1 per channel)

Strategy:
  - Data-parallel over batch: 32 images -> 4 per core across 8 cores.
  - Convs as offset-accumulated matmuls on the tensor engine in
    float32r (FP22 multiplies, fp32 accumulate, 1 column/cycle at
    N=512).
  - Contraction = input channels. 144 = 128 (chunk A, kh/kw shifts
    via padded row/column slicing of a [128,70,70] image) + 16
    (chunk B: 7 column-shifted copies x 16 channels packed on 112
    partitions so ONE matmul covers every kw offset of a kernel row).
  - Images are padded/pre-shifted on the host so each batch needs
    exactly one DMA per chunk (keeps matmul sync-wait counts tiny).
  - BN+ReLU folded into a single scalar-engine activation on PSUM
    eviction (scale/bias precomputed on host; conv bias folded in).
  - avg folded into the fusion conv weights: out = conv(add, wa) +
    conv(mx, wm) with wa = wf[0]+wf[1]/3, wm = wf[2]; depthwise 3x3
    computed on the vector engine as 18 scalar_tensor_tensor MACs.
"""

import json
import numpy as np
from contextlib import ExitStack

import concourse.bass as bass
import concourse.tile as tile
from concourse import mybir
from concourse.bass_utils import run_bass_kernel_spmd


def _legalize_sync_waits(raw: bytes) -> bytes:
    """Split multi-wait instructions for a walrus that accepts only one
    sync wait per instruction.

    The Tile scheduler emits sync_info.on_wait lists with up to ~11
    entries, but this neuronxcc's codegen rejects >1 wait per
    instruction ("Too many sync wait commands"). Keep the last wait on
    the instruction and hoist the rest into standalone EventSemaphore
    instructions placed immediately before it in the same engine
    stream. This is semantics-preserving for compute engines (they
    would have blocked at the instruction anyway) and safe for DMAs
    here because emission order is a topological order of the
    dependency graph (DMA n's waits only ever point at compute over
    earlier DMAs' data).
    """
    d = json.loads(raw)
    ctr = 0
    for fn in d["functions"]:
        for bb in fn["blocks"]:
            out = []
            for inst in bb["instructions"]:
                si = inst.get("sync_info")
                if si:
                    ow = si.get("on_wait") or []
                    if len(ow) > 1:
                        for w in ow[:-1]:
                            ctr += 1
                            out.append({
                                "debug": inst.get("debug"),
                                "engine": inst["engine"],
                                "ins": [], "outs": [],
                                "name": f"waitsplit-{ctr}",
                                "opcode": "EventSemaphore",
                                "sync_info": {"on_update": [], "on_wait": [w]},
                            })
                        si["on_wait"] = [ow[-1]]
                out.append(inst)
            bb["instructions"] = out
    return json.dumps(d).encode()


class _LegalizedBass(bass.Bass):
    def to_json_bytes(self, *a, **kw):
        return _legalize_sync_waits(super().to_json_bytes(*a, **kw))

F32 = mybir.dt.float32
F32R = mybir.dt.float32r
F16 = mybir.dt.float16
ALU = mybir.AluOpType
ACTF = mybir.ActivationFunctionType

N_CORES = 8
B_LOC = 4                  # batch images per core
L, D, H, W = 144, 96, 64, 64
PAD = 3                    # global padding used in SBUF image tiles
HP, WP = H + 2 * PAD, W + 2 * PAD   # 70, 70
RB = 8                     # output rows per PSUM block
NBLK = H // RB             # 8 blocks
CONVS = [(3, 1), (5, 2), (7, 3)]    # (kernel, pad) per branch
ORDER = [3, 2, 4, 1, 5, 0, 6]       # chunk-B kw-shift groups, center-out
A_BASE = [0, 9, 34]        # chunk-A (kh,kw)-offset index base per conv
B_BASE = [0, 3, 8]         # chunk-B kh-column base per conv
N_OFF_A = 83               # 9 + 25 + 49
N_COL_B = 15               # 3 + 5 + 7
EPS = 1e-5

_cache = {}


def _build_program(iters=1, variant='full'):
    nc = _LegalizedBass()
    xa_p = nc.declare_dram_parameter("xap", [B_LOC, 128, HP, WP], F16, isOutput=False)
    xb_p = nc.declare_dram_parameter("xbp", [B_LOC, 112, HP, W], F16, isOutput=False)
    wa_p = nc.declare_dram_parameter("wa", [128, N_OFF_A * D], F16, isOutput=False)
    wb_p = nc.declare_dram_parameter("wb", [112, N_COL_B * D], F16, isOutput=False)
    fw_p = nc.declare_dram_parameter("fw", [D, 18], F32, isOutput=False)
    sc_p = nc.declare_dram_parameter("sc", [D, 6], F32, isOutput=False)
    out_p = nc.declare_dram_parameter("out", [B_LOC, D, H, W], F32, isOutput=True)

    with tile.TileContext(nc) as tc, ExitStack() as ctx:
        cpool = ctx.enter_context(tc.tile_pool(name="const", bufs=1))
        spool = ctx.enter_context(tc.tile_pool(name="stage", bufs=2))
        opool = ctx.enter_context(tc.tile_pool(name="outp", bufs=1))
        ppool = ctx.enter_context(tc.tile_pool(name="psum", bufs=2, space="PSUM"))

        wa_t = cpool.tile([128, N_OFF_A * D], F16, tag="wa")
        nc.sync.dma_start(wa_t[:], wa_p[:])
        wb_t = cpool.tile([112, N_COL_B * D], F16, tag="wb")
        nc.sync.dma_start(wb_t[:], wb_p[:])
        fw_t = cpool.tile([D, 18], F32, tag="fw")
        nc.sync.dma_start(fw_t[:], fw_p[:])
        sc_t = cpool.tile([D, 6], F32, tag="sc")
        nc.sync.dma_start(sc_t[:], sc_p[:])

        # Double-buffered (by batch parity) padded input images and
        # padded add/max accumulators. Image padding/shifting is done on
        # the host; accumulator borders are zeroed once here (interiors
        # are fully rewritten per batch).
        xa = [cpool.tile([128, HP, WP], F16, tag=f"xa{i}", name=f"xa{i}")
              for i in range(2)]
        xb = [cpool.tile([112, HP, W], F16, tag=f"xb{i}", name=f"xb{i}")
              for i in range(2)]
        addp = [cpool.tile([D, H + 2, W + 2], F32, tag=f"addp{i}", name=f"addp{i}")
                for i in range(2)]
        mxp = [cpool.tile([D, H + 2, W + 2], F32, tag=f"mxp{i}", name=f"mxp{i}")
               for i in range(2)]
        for t in addp + mxp:
            nc.vector.memset(t[:], 0.0)

        for b in range(B_LOC * iters):
            b = b % B_LOC
            xat, xbt = xa[b % 2], xb[b % 2]
            ap, mp = addp[b % 2], mxp[b % 2]

            nc.sync.dma_start(xat[:], xa_p[b])
            nc.sync.dma_start(xbt[:], xb_p[b])

            for blk in range(NBLK):
                h0 = blk * RB
                stg = []
                for ci, (k, p) in enumerate(CONVS):
                    ps = ppool.tile([D, RB, W], F32, tag=f"ps{ci}")
                    kh_list = list(range(k))
                    if variant == 'fewmm' and k > 3:
                        kh_list = [p]
                    n_mm = (k + 1) * len(kh_list)
                    mi = 0
                    for kh in kh_list:
                        r0 = h0 + kh - p + PAD
                        for kw in range(k):
                            c0 = kw - p + PAD
                            aoff = (A_BASE[ci] + kh * k + kw) * D
                            nc.tensor.matmul(
                                ps[:],
                                wa_t[:, aoff:aoff + D],
                                xat[:, r0:r0 + RB, c0:c0 + W],
                                start=(mi == 0), stop=(mi == n_mm - 1),
                            )
                            mi += 1
                        kb = 16 * (2 * p + 1)
                        boff = (B_BASE[ci] + kh) * D
                        nc.tensor.matmul(
                            ps[:],
                            wb_t[0:kb, boff:boff + D],
                            xbt[0:kb, r0:r0 + RB, 0:W],
                            start=False, stop=(mi == n_mm - 1),
                        )
                        mi += 1
                    stg.append(ps)

                # BN+ReLU eviction. conv3 is evicted twice: directly into
                # the add and max accumulators; conv1/conv2 go to staging
                # and are combined in-place on the vector engine.
                av = ap[:, 1 + h0:1 + h0 + RB, 1:1 + W]
                mv = mp[:, 1 + h0:1 + h0 + RB, 1:1 + W]
                x1b = spool.tile([D, RB, W], F32, tag="x1b")
                x2b = spool.tile([D, RB, W], F32, tag="x2b")
                nc.scalar.activation(x1b[:], stg[0][:], ACTF.Relu,
                                     bias=sc_t[:, 1:2], scale=sc_t[:, 0:1])
                nc.scalar.activation(x2b[:], stg[1][:], ACTF.Relu,
                                     bias=sc_t[:, 3:4], scale=sc_t[:, 2:3])
                nc.scalar.activation(av, stg[2][:], ACTF.Relu,
                                     bias=sc_t[:, 5:6], scale=sc_t[:, 4:5])
                nc.scalar.activation(mv, stg[2][:], ACTF.Relu,
                                     bias=sc_t[:, 5:6], scale=sc_t[:, 4:5])
                if variant not in ('novec',):
                    nc.vector.tensor_tensor(av, av, x1b[:], ALU.add)
                    nc.vector.tensor_tensor(av, av, x2b[:], ALU.add)
                    nc.vector.tensor_tensor(mv, mv, x1b[:], ALU.max)
                    nc.vector.tensor_tensor(mv, mv, x2b[:], ALU.max)

            # depthwise 3x3 fusion conv: out = conv(add, wa) + conv(mx, wm)
            ob = opool.tile([D, H, W], F32, tag="outb")
            if variant in ('nofuse', 'novec'):
                nc.vector.tensor_copy(ob[:], ap[:, 0:H, 0:W])
                nc.sync.dma_start(out_p[b], ob[:])
                continue
            idx = 0
            for src, fbase in ((ap, 0), (mp, 9)):
                for dh in range(3):
                    for dw in range(3):
                        iv = src[:, dh:dh + H, dw:dw + W]
                        w_ap = fw_t[:, fbase + dh * 3 + dw:fbase + dh * 3 + dw + 1]
                        if idx == 0:
                            nc.vector.tensor_scalar_mul(ob[:], iv, w_ap)
                        else:
                            nc.vector.scalar_tensor_tensor(
                                ob[:], iv, w_ap, ob[:], ALU.mult, ALU.add)
                        idx += 1
            nc.sync.dma_start(out_p[b], ob[:])

    return nc


def _host_prep(inputs):
    """Rearrange weights / fold BN on the host (numpy only)."""
    w = [np.asarray(inputs[n], np.float32) for n in ("w1", "w2", "w3")]
    wa = np.zeros((128, N_OFF_A * D), np.float16)
    idx = 0
    for wi, (k, p) in zip(w, CONVS):
        for kh in range(k):
            for kw in range(k):
                wa[:, idx * D:(idx + 1) * D] = wi[:, :128, kh, kw].T
                idx += 1
    wb = np.zeros((112, N_COL_B * D), np.float16)
    col = 0
    for wi, (k, p) in zip(w, CONVS):
        for kh in range(k):
            for g in range(2 * p + 1):
                kw = ORDER[g] - 3 + p
                wb[g * 16:(g + 1) * 16, col * D:(col + 1) * D] = wi[:, 128:L, kh, kw].T
            col += 1

    sc = np.zeros((D, 6), np.float32)
    for ci, pre in enumerate(("1", "2", "3")):
        g = np.asarray(inputs["g" + pre], np.float32)
        be = np.asarray(inputs["be" + pre], np.float32)
        m = np.asarray(inputs["m" + pre], np.float32)
        v = np.asarray(inputs["v" + pre], np.float32)
        bconv = np.asarray(inputs["b" + pre], np.float32)
        inv = g / np.sqrt(v + EPS)
        sc[:, 2 * ci] = inv
        sc[:, 2 * ci + 1] = bconv * inv + be - m * inv

    wf = np.asarray(inputs["wf"], np.float32)
    wfa = wf[0, 0] + wf[0, 1] / 3.0    # [3,3]
    wfm = wf[0, 2]
    fw = np.zeros((D, 18), np.float32)
    fw[:, 0:9] = wfa.reshape(1, 9)
    fw[:, 9:18] = wfm.reshape(1, 9)
    return wa, wb, fw, sc


def _pad_images(x):
    """Host-side padding/pre-shifting of the full input batch."""
    nb = x.shape[0]
    xap = np.zeros((nb, 128, HP, WP), np.float16)
    xap[:, :, PAD:PAD + H, PAD:PAD + W] = x[:, :128]
    xbp = np.zeros((nb, 112, HP, W), np.float16)
    for g in range(7):
        s = ORDER[g] - 3
        c0, c1 = max(0, -s), W - max(0, s)
        xbp[:, g * 16:(g + 1) * 16, PAD:PAD + H, c0:c1] = x[:, 128:L, :, c0 + s:c1 + s]
    return xap, xbp


def _run(inputs, iters=1, variant='full'):
    key = f"nc{iters}-{variant}"
    if key not in _cache:
        _cache[key] = _build_program(iters, variant)
    nc = _cache[key]

    x = np.ascontiguousarray(np.asarray(inputs["x"], np.float32))
    wa, wb, fw, sc = _host_prep(inputs)
    xap, xbp = _pad_images(x)

    core_ids = list(range(N_CORES))
    in_maps = []
    for c in core_ids:
        in_maps.append({
            "xap": xap[c * B_LOC:(c + 1) * B_LOC],
            "xbp": xbp[c * B_LOC:(c + 1) * B_LOC],
            "wa": wa, "wb": wb, "fw": fw, "sc": sc,
        })
    res = run_bass_kernel_spmd(nc, in_maps, core_ids)
    outs = [np.asarray(res.results[i]["out"]) for i in range(N_CORES)]
    return np.concatenate(outs, axis=0)


def kernel(**inputs):
    return _run(inputs, iters=1)



# revision 2
# speedup vs baseline: 517.9211x; 517.9211x over previous
"""Trainium2 Bass kernel for nn_CNN_Encoder_33328946217622.

Reference computation (per batch image):
  x: [144, 64, 64]
  x1 = relu(BN1(conv3x3(x, w1) + b1))     (96 ch)
  x2 = relu(BN2(conv5x5(x, w2) + b2))
  x3 = relu(BN3(conv7x7(x, w3) + b3))
  add = x1+x2+x3 ; avg = add/3 ; mx = max(x1,x2,x3)
  out = conv3x3_depthwise_shared([add, avg, mx], wf)   (3->1 per channel)

Strategy:
  - Data-parallel over batch: 32 images -> 4 per core across 8 cores.
  - Convs as offset-accumulated matmuls on the tensor engine (fp16
    inputs, fp32 PSUM accumulate).
  - Contraction = input channels. 144 = 128 (chunk A, kh/kw shifts
    via padded row/column slicing of a [128,70,70] image) + 16
    (chunk B: 7 column-shifted copies x 16 channels packed on 112
    partitions so ONE matmul covers every kw offset of a kernel row).
  - Images are padded/pre-shifted on the host so each batch needs
    exactly one DMA per chunk.
  - The per-core batch (and the test harness's K-loop repetition) runs
    in HARDWARE For_i loops with dynamic DRAM offsets, so the static
    instruction stream is one image's worth (~1.7k instructions)
    regardless of iteration count: per-dispatch NEFF processing
    overhead (which dominates in this environment) stays constant and
    small.
  - BN+ReLU folded into a single scalar-engine activation on PSUM
    eviction (scale/bias precomputed on host; conv bias folded in).
  - avg folded into the fusion conv weights: out = conv(add, wa) +
    conv(mx, wm) with wa = wf[0]+wf[1]/3, wm = wf[2]; depthwise 3x3
    computed on the vector engine as 18 scalar_tensor_tensor MACs.
"""

import json
import numpy as np
from contextlib import ExitStack

import concourse.bass as bass
import concourse.tile as tile
from concourse import mybir
from concourse.bass import ts
from concourse.bass_utils import run_bass_kernel_spmd


def _legalize_sync_waits(raw: bytes) -> bytes:
    """Split multi-wait instructions for a walrus that accepts only one
    sync wait per instruction.

    The Tile scheduler emits sync_info.on_wait lists with up to ~11
    entries, but this neuronxcc's codegen rejects >1 wait per
    instruction ("Too many sync wait commands"). Keep the last wait on
    the instruction and hoist the rest into standalone EventSemaphore
    instructions placed immediately before it in the same engine
    stream. This is semantics-preserving for compute engines (they
    would have blocked at the instruction anyway) and safe for DMAs
    here because emission order is a topological order of the
    dependency graph (DMA n's waits only ever point at compute over
    earlier DMAs' data).
    """
    d = json.loads(raw)
    ctr = 0
    for fn in d["functions"]:
        for bb in fn["blocks"]:
            out = []
            for inst in bb["instructions"]:
                si = inst.get("sync_info")
                if si:
                    ow = si.get("on_wait") or []
                    if len(ow) > 1:
                        for w in ow[:-1]:
                            ctr += 1
                            out.append({
                                "debug": inst.get("debug"),
                                "engine": inst["engine"],
                                "ins": [], "outs": [],
                                "name": f"waitsplit-{ctr}",
                                "opcode": "EventSemaphore",
                                "sync_info": {"on_update": [], "on_wait": [w]},
                            })
                        si["on_wait"] = [ow[-1]]
                out.append(inst)
            bb["instructions"] = out
    return json.dumps(d).encode()


class _LegalizedBass(bass.Bass):
    def to_json_bytes(self, *a, **kw):
        return _legalize_sync_waits(super().to_json_bytes(*a, **kw))

F32 = mybir.dt.float32
F16 = mybir.dt.float16
ALU = mybir.AluOpType
ACTF = mybir.ActivationFunctionType

N_CORES = 8
B_LOC = 4                  # batch images per core
L, D, H, W = 144, 96, 64, 64
PAD = 3                    # global padding used in SBUF image tiles
HP, WP = H + 2 * PAD, W + 2 * PAD   # 70, 70
RB = 8                     # output rows per PSUM block
NBLK = H // RB             # 8 blocks
CONVS = [(3, 1), (5, 2), (7, 3)]    # (kernel, pad) per branch
ORDER = [3, 2, 4, 1, 5, 0, 6]       # chunk-B kw-shift groups, center-out
A_BASE = [0, 9, 34]        # chunk-A (kh,kw)-offset index base per conv
B_BASE = [0, 3, 8]         # chunk-B kh-column base per conv
N_OFF_A = 83               # 9 + 25 + 49
N_COL_B = 15               # 3 + 5 + 7
EPS = 1e-5

_cache = {}


def _build_program(iters=1, variant='full'):
    nc = _LegalizedBass()
    xa_p = nc.declare_dram_parameter("xap", [B_LOC * 128, HP, WP], F16, isOutput=False)
    xb_p = nc.declare_dram_parameter("xbp", [B_LOC * 112, HP, W], F16, isOutput=False)
    wa_p = nc.declare_dram_parameter("wa", [128, N_OFF_A * D], F16, isOutput=False)
    wb_p = nc.declare_dram_parameter("wb", [112, N_COL_B * D], F16, isOutput=False)
    fw_p = nc.declare_dram_parameter("fw", [D, 18], F32, isOutput=False)
    sc_p = nc.declare_dram_parameter("sc", [D, 6], F32, isOutput=False)
    out_p = nc.declare_dram_parameter("out", [B_LOC * D, H, W], F32, isOutput=True)

    with tile.TileContext(nc) as tc, ExitStack() as ctx:
        cpool = ctx.enter_context(tc.tile_pool(name="const", bufs=1))
        spool = ctx.enter_context(tc.tile_pool(name="stage", bufs=2))
        opool = ctx.enter_context(tc.tile_pool(name="outp", bufs=1))
        ppool = ctx.enter_context(tc.tile_pool(name="psum", bufs=2, space="PSUM"))

        wa_t = cpool.tile([128, N_OFF_A * D], F16, tag="wa")
        nc.sync.dma_start(wa_t[:], wa_p[:])
        wb_t = cpool.tile([112, N_COL_B * D], F16, tag="wb")
        nc.sync.dma_start(wb_t[:], wb_p[:])
        fw_t = cpool.tile([D, 18], F32, tag="fw")
        nc.sync.dma_start(fw_t[:], fw_p[:])
        sc_t = cpool.tile([D, 6], F32, tag="sc")
        nc.sync.dma_start(sc_t[:], sc_p[:])

        # Padded input image and padded add/max accumulators. Image
        # padding/shifting is done on the host; accumulator borders are
        # zeroed once here (interiors are fully rewritten per batch).
        xat = cpool.tile([128, HP, WP], F16, tag="xa", name="xa")
        xbt = cpool.tile([112, HP, W], F16, tag="xb", name="xb")
        ap = cpool.tile([D, H + 2, W + 2], F32, tag="addp", name="addp")
        mp = cpool.tile([D, H + 2, W + 2], F32, tag="mxp", name="mxp")
        nc.vector.memset(ap[:], 0.0)
        nc.vector.memset(mp[:], 0.0)

        with tc.For_i(0, iters, 1, name="rep"):
            with tc.For_i(0, B_LOC, 1, name="img") as b:
                nc.sync.dma_start(xat[:], xa_p[ts(b, 128)])
                nc.sync.dma_start(xbt[:], xb_p[ts(b, 112)])

                for blk in range(NBLK):
                    h0 = blk * RB
                    stg = []
                    for ci, (k, p) in enumerate(CONVS):
                        ps = ppool.tile([D, RB, W], F32, tag=f"ps{ci}")
                        kh_list = list(range(k))
                        if variant == 'fewmm' and k > 3:
                            kh_list = [p]
                        n_mm = (k + 1) * len(kh_list)
                        mi = 0
                        for kh in kh_list:
                            r0 = h0 + kh - p + PAD
                            for kw in range(k):
                                c0 = kw - p + PAD
                                aoff = (A_BASE[ci] + kh * k + kw) * D
                                nc.tensor.matmul(
                                    ps[:],
                                    wa_t[:, aoff:aoff + D],
                                    xat[:, r0:r0 + RB, c0:c0 + W],
                                    start=(mi == 0), stop=(mi == n_mm - 1),
                                )
                                mi += 1
                            kb = 16 * (2 * p + 1)
                            boff = (B_BASE[ci] + kh) * D
                            nc.tensor.matmul(
                                ps[:],
                                wb_t[0:kb, boff:boff + D],
                                xbt[0:kb, r0:r0 + RB, 0:W],
                                start=False, stop=(mi == n_mm - 1),
                            )
                            mi += 1
                        stg.append(ps)

                    # BN+ReLU eviction. conv3 is evicted twice: directly
                    # into the add and max accumulators; conv1/conv2 go to
                    # staging and are combined in-place on the vector
                    # engine.
                    av = ap[:, 1 + h0:1 + h0 + RB, 1:1 + W]
                    mv = mp[:, 1 + h0:1 + h0 + RB, 1:1 + W]
                    x1b = spool.tile([D, RB, W], F32, tag="x1b")
                    x2b = spool.tile([D, RB, W], F32, tag="x2b")
                    nc.scalar.activation(x1b[:], stg[0][:], ACTF.Relu,
                                         bias=sc_t[:, 1:2], scale=sc_t[:, 0:1])
                    nc.scalar.activation(x2b[:], stg[1][:], ACTF.Relu,
                                         bias=sc_t[:, 3:4], scale=sc_t[:, 2:3])
                    nc.scalar.activation(av, stg[2][:], ACTF.Relu,
                                         bias=sc_t[:, 5:6], scale=sc_t[:, 4:5])
                    nc.scalar.activation(mv, stg[2][:], ACTF.Relu,
                                         bias=sc_t[:, 5:6], scale=sc_t[:, 4:5])
                    if variant not in ('novec',):
                        nc.vector.tensor_tensor(av, av, x1b[:], ALU.add)
                        nc.vector.tensor_tensor(av, av, x2b[:], ALU.add)
                        nc.vector.tensor_tensor(mv, mv, x1b[:], ALU.max)
                        nc.vector.tensor_tensor(mv, mv, x2b[:], ALU.max)

                # depthwise 3x3 fusion: out = conv(add, wa) + conv(mx, wm)
                ob = opool.tile([D, H, W], F32, tag="outb")
                if variant in ('nofuse', 'novec'):
                    nc.vector.tensor_copy(ob[:], ap[:, 0:H, 0:W])
                    nc.sync.dma_start(out_p[ts(b, D)], ob[:])
                else:
                    idx = 0
                    for src, fbase in ((ap, 0), (mp, 9)):
                        for dh in range(3):
                            for dw in range(3):
                                iv = src[:, dh:dh + H, dw:dw + W]
                                fo = fbase + dh * 3 + dw
                                w_ap = fw_t[:, fo:fo + 1]
                                if idx == 0:
                                    nc.vector.tensor_scalar_mul(ob[:], iv, w_ap)
                                else:
                                    nc.vector.scalar_tensor_tensor(
                                        ob[:], iv, w_ap, ob[:], ALU.mult, ALU.add)
                                idx += 1
                    nc.sync.dma_start(out_p[ts(b, D)], ob[:])

    return nc


def _host_prep(inputs):
    """Rearrange weights / fold BN on the host (numpy only)."""
    w = [np.asarray(inputs[n], np.float32) for n in ("w1", "w2", "w3")]
    wa = np.zeros((128, N_OFF_A * D), np.float16)
    idx = 0
    for wi, (k, p) in zip(w, CONVS):
        for kh in range(k):
            for kw in range(k):
                wa[:, idx * D:(idx + 1) * D] = wi[:, :128, kh, kw].T
                idx += 1
    wb = np.zeros((112, N_COL_B * D), np.float16)
    col = 0
    for wi, (k, p) in zip(w, CONVS):
        for kh in range(k):
            for g in range(2 * p + 1):
                kw = ORDER[g] - 3 + p
                wb[g * 16:(g + 1) * 16, col * D:(col + 1) * D] = wi[:, 128:L, kh, kw].T
            col += 1

    sc = np.zeros((D, 6), np.float32)
    for ci, pre in enumerate(("1", "2", "3")):
        g = np.asarray(inputs["g" + pre], np.float32)
        be = np.asarray(inputs["be" + pre], np.float32)
        m = np.asarray(inputs["m" + pre], np.float32)
        v = np.asarray(inputs["v" + pre], np.float32)
        bconv = np.asarray(inputs["b" + pre], np.float32)
        inv = g / np.sqrt(v + EPS)
        sc[:, 2 * ci] = inv
        sc[:, 2 * ci + 1] = bconv * inv + be - m * inv

    wf = np.asarray(inputs["wf"], np.float32)
    wfa = wf[0, 0] + wf[0, 1] / 3.0    # [3,3]
    wfm = wf[0, 2]
    fw = np.zeros((D, 18), np.float32)
    fw[:, 0:9] = wfa.reshape(1, 9)
    fw[:, 9:18] = wfm.reshape(1, 9)
    return wa, wb, fw, sc


def _pad_images(x):
    """Host-side padding/pre-shifting of the full input batch."""
    nb = x.shape[0]
    xap = np.zeros((nb, 128, HP, WP), np.float16)
    xap[:, :, PAD:PAD + H, PAD:PAD + W] = x[:, :128]
    xbp = np.zeros((nb, 112, HP, W), np.float16)
    for g in range(7):
        s = ORDER[g] - 3
        c0, c1 = max(0, -s), W - max(0, s)
        xbp[:, g * 16:(g + 1) * 16, PAD:PAD + H, c0:c1] = x[:, 128:L, :, c0 + s:c1 + s]
    return xap, xbp


def _run(inputs, iters=1, variant='full'):
    key = f"nc{iters}-{variant}"
    if key not in _cache:
        _cache[key] = _build_program(iters, variant)
    nc = _cache[key]

    x = np.ascontiguousarray(np.asarray(inputs["x"], np.float32))
    wa, wb, fw, sc = _host_prep(inputs)
    xap, xbp = _pad_images(x)
    xap = xap.reshape(N_CORES, B_LOC * 128, HP, WP)
    xbp = xbp.reshape(N_CORES, B_LOC * 112, HP, W)

    core_ids = list(range(N_CORES))
    in_maps = []
    for c in core_ids:
        in_maps.append({
            "xap": xap[c],
            "xbp": xbp[c],
            "wa": wa, "wb": wb, "fw": fw, "sc": sc,
        })
    res = run_bass_kernel_spmd(nc, in_maps, core_ids)
    outs = [np.asarray(res.results[i]["out"]).reshape(B_LOC, D, H, W)
            for i in range(N_CORES)]
    return np.concatenate(outs, axis=0)


def kernel(**inputs):
    return _run(inputs, iters=1)


# revision 5
# speedup vs baseline: 627.9243x; 1.2124x over previous
"""Trainium2 Bass kernel for nn_CNN_Encoder_33328946217622.

Reference computation (per batch image):
  x: [144, 64, 64]
  x1 = relu(BN1(conv3x3(x, w1) + b1))     (96 ch)
  x2 = relu(BN2(conv5x5(x, w2) + b2))
  x3 = relu(BN3(conv7x7(x, w3) + b3))
  add = x1+x2+x3 ; avg = add/3 ; mx = max(x1,x2,x3)
  out = conv3x3_depthwise_shared([add, avg, mx], wf)   (3->1 per channel)

Strategy:
  - Data-parallel over batch: 32 images -> 4 per core across 8 cores.
  - Convs as offset-accumulated matmuls on the tensor engine (fp16
    inputs, fp32 PSUM accumulate).
  - Contraction = input channels. 144 = 128 (chunk A, kh/kw shifts
    via padded row/column slicing of a [128,70,70] image) + 16
    (chunk B: 7 column-shifted copies x 16 channels packed on 112
    partitions so ONE matmul covers every kw offset of a kernel row).
  - Images are padded/pre-shifted on the host so each batch needs
    exactly one DMA per chunk.
  - The per-core batch (and the test harness's K-loop repetition) runs
    in HARDWARE For_i loops with dynamic DRAM offsets, so the static
    instruction stream is one image's worth (~1.7k instructions)
    regardless of iteration count: per-dispatch NEFF processing
    overhead (which dominates in this environment) stays constant and
    small.
  - BN+ReLU folded into a single scalar-engine activation on PSUM
    eviction (scale/bias precomputed on host; conv bias folded in).
  - avg folded into the fusion conv weights: out = conv(add, wa) +
    conv(mx, wm) with wa = wf[0]+wf[1]/3, wm = wf[2]; depthwise 3x3
    computed on the vector engine as 18 scalar_tensor_tensor MACs.
"""

import json
import numpy as np
from contextlib import ExitStack

import concourse.bass as bass
import concourse.tile as tile
from concourse import mybir
from concourse.bass import ts
from concourse.bass_utils import run_bass_kernel_spmd


def _legalize_sync_waits(raw: bytes) -> bytes:
    """Split multi-wait instructions for a walrus that accepts only one
    sync wait per instruction.

    The Tile scheduler emits sync_info.on_wait lists with up to ~11
    entries, but this neuronxcc's codegen rejects >1 wait per
    instruction ("Too many sync wait commands"). Keep the last wait on
    the instruction and hoist the rest into standalone EventSemaphore
    instructions placed immediately before it in the same engine
    stream. This is semantics-preserving for compute engines (they
    would have blocked at the instruction anyway) and safe for DMAs
    here because emission order is a topological order of the
    dependency graph (DMA n's waits only ever point at compute over
    earlier DMAs' data).
    """
    d = json.loads(raw)
    ctr = 0
    for fn in d["functions"]:
        for bb in fn["blocks"]:
            out = []
            for inst in bb["instructions"]:
                si = inst.get("sync_info")
                if si:
                    ow = si.get("on_wait") or []
                    if len(ow) > 1:
                        for w in ow[:-1]:
                            ctr += 1
                            out.append({
                                "debug": inst.get("debug"),
                                "engine": inst["engine"],
                                "ins": [], "outs": [],
                                "name": f"waitsplit-{ctr}",
                                "opcode": "EventSemaphore",
                                "sync_info": {"on_update": [], "on_wait": [w]},
                            })
                        si["on_wait"] = [ow[-1]]
                out.append(inst)
            bb["instructions"] = out
    return json.dumps(d).encode()


class _LegalizedBass(bass.Bass):
    def to_json_bytes(self, *a, **kw):
        return _legalize_sync_waits(super().to_json_bytes(*a, **kw))

F32 = mybir.dt.float32
F16 = mybir.dt.float16
ALU = mybir.AluOpType
ACTF = mybir.ActivationFunctionType

N_CORES = 8
B_LOC = 4                  # batch images per core
L, D, H, W = 144, 96, 64, 64
PAD = 3                    # global padding used in SBUF image tiles
HP, WP = H + 2 * PAD, W + 2 * PAD   # 70, 70
RB = 8                     # output rows per PSUM block
NBLK = H // RB             # 8 blocks
CONVS = [(3, 1), (5, 2), (7, 3)]    # (kernel, pad) per branch
ORDER = [3, 2, 4, 1, 5, 0, 6]       # chunk-B kw-shift groups, center-out
A_BASE = [0, 9, 34]        # chunk-A (kh,kw)-offset index base per conv
B_BASE = [0, 3, 8]         # chunk-B kh-column base per conv
N_OFF_A = 83               # 9 + 25 + 49
N_COL_B = 15               # 3 + 5 + 7
EPS = 1e-5

_cache = {}


def _build_program(iters=1, variant='full'):
    nc = _LegalizedBass()
    xa_p = nc.declare_dram_parameter("xap", [B_LOC * 128, HP, WP], F16, isOutput=False)
    xb_p = nc.declare_dram_parameter("xbp", [B_LOC * 112, HP, W], F16, isOutput=False)
    wa_p = nc.declare_dram_parameter("wa", [128, N_OFF_A * D], F16, isOutput=False)
    wb_p = nc.declare_dram_parameter("wb", [112, N_COL_B * D], F16, isOutput=False)
    fw_p = nc.declare_dram_parameter("fw", [D, 18], F32, isOutput=False)
    sc_p = nc.declare_dram_parameter("sc", [D, 6], F32, isOutput=False)
    out_p = nc.declare_dram_parameter("out", [B_LOC * D, H, W], F32, isOutput=True)

    with tile.TileContext(nc) as tc, ExitStack() as ctx:
        cpool = ctx.enter_context(tc.tile_pool(name="const", bufs=1))
        spool = ctx.enter_context(tc.tile_pool(name="stage", bufs=2))
        opool = ctx.enter_context(tc.tile_pool(name="outp", bufs=1))
        ppool = ctx.enter_context(tc.tile_pool(name="psum", bufs=2, space="PSUM"))

        wa_t = cpool.tile([128, N_OFF_A * D], F16, tag="wa")
        nc.sync.dma_start(wa_t[:], wa_p[:])
        wb_t = cpool.tile([112, N_COL_B * D], F16, tag="wb")
        nc.sync.dma_start(wb_t[:], wb_p[:])
        fw_t = cpool.tile([D, 18], F32, tag="fw")
        nc.sync.dma_start(fw_t[:], fw_p[:])
        sc_t = cpool.tile([D, 6], F32, tag="sc")
        nc.sync.dma_start(sc_t[:], sc_p[:])

        # Double-buffered (image parity within a pair) padded input
        # images and padded add/max accumulators. Image padding/shifting
        # is done on the host; accumulator borders are zeroed once here
        # (interiors are fully rewritten per batch). Processing images
        # in pairs lets image A's vector-engine fusion overlap image B's
        # tensor-engine matmuls.
        xa = [cpool.tile([128, HP, WP], F16, tag=f"xa{i}", name=f"xa{i}")
              for i in range(2)]
        xb = [cpool.tile([112, HP, W], F16, tag=f"xb{i}", name=f"xb{i}")
              for i in range(2)]
        addp = [cpool.tile([D, H + 2, W + 2], F32, tag=f"addp{i}", name=f"addp{i}")
                for i in range(2)]
        mxp = [cpool.tile([D, H + 2, W + 2], F32, tag=f"mxp{i}", name=f"mxp{i}")
               for i in range(2)]
        for t in addp + mxp:
            nc.vector.memset(t[:], 0.0)

        with tc.For_i(0, iters, 1, name="rep"), \
             tc.For_i(0, B_LOC // 2, 1, name="pair") as pr:
            xa_pr = xa_p[ts(pr, 2 * 128)]
            xb_pr = xb_p[ts(pr, 2 * 112)]
            out_pr = out_p[ts(pr, 2 * D)]
            for half in range(2):
                xat, xbt = xa[half], xb[half]
                ap, mp = addp[half], mxp[half]
                nc.sync.dma_start(xat[:], xa_pr[half * 128:half * 128 + 128])
                nc.sync.dma_start(xbt[:], xb_pr[half * 112:half * 112 + 112])

                for blk in range(NBLK):
                    h0 = blk * RB
                    stg = []
                    for ci, (k, p) in enumerate(CONVS):
                        ps = ppool.tile([D, RB, W], F32, tag=f"ps{ci}")
                        kh_list = list(range(k))
                        if variant == 'fewmm' and k > 3:
                            kh_list = [p]
                        n_mm = (k + 1) * len(kh_list)
                        mi = 0
                        for kh in kh_list:
                            r0 = h0 + kh - p + PAD
                            for kw in range(k):
                                c0 = kw - p + PAD
                                aoff = (A_BASE[ci] + kh * k + kw) * D
                                nc.tensor.matmul(
                                    ps[:],
                                    wa_t[:, aoff:aoff + D],
                                    xat[:, r0:r0 + RB, c0:c0 + W],
                                    start=(mi == 0), stop=(mi == n_mm - 1),
                                )
                                mi += 1
                            kb = 16 * (2 * p + 1)
                            boff = (B_BASE[ci] + kh) * D
                            nc.tensor.matmul(
                                ps[:],
                                wb_t[0:kb, boff:boff + D],
                                xbt[0:kb, r0:r0 + RB, 0:W],
                                start=False, stop=(mi == n_mm - 1),
                            )
                            mi += 1
                        stg.append(ps)

                    # BN+ReLU eviction. conv3 is evicted twice: directly
                    # into the add and max accumulators; conv1/conv2 go to
                    # staging and are combined in-place on the vector
                    # engine.
                    av = ap[:, 1 + h0:1 + h0 + RB, 1:1 + W]
                    mv = mp[:, 1 + h0:1 + h0 + RB, 1:1 + W]
                    x1b = spool.tile([D, RB, W], F32, tag="x1b")
                    x2b = spool.tile([D, RB, W], F32, tag="x2b")
                    nc.scalar.activation(x1b[:], stg[0][:], ACTF.Relu,
                                         bias=sc_t[:, 1:2], scale=sc_t[:, 0:1])
                    nc.scalar.activation(x2b[:], stg[1][:], ACTF.Relu,
                                         bias=sc_t[:, 3:4], scale=sc_t[:, 2:3])
                    nc.scalar.activation(av, stg[2][:], ACTF.Relu,
                                         bias=sc_t[:, 5:6], scale=sc_t[:, 4:5])
                    nc.scalar.activation(mv, stg[2][:], ACTF.Relu,
                                         bias=sc_t[:, 5:6], scale=sc_t[:, 4:5])
                    if variant not in ('novec',):
                        nc.vector.tensor_tensor(av, av, x1b[:], ALU.add)
                        nc.vector.tensor_tensor(av, av, x2b[:], ALU.add)
                        nc.vector.tensor_tensor(mv, mv, x1b[:], ALU.max)
                        nc.vector.tensor_tensor(mv, mv, x2b[:], ALU.max)

                # depthwise 3x3 fusion: out = conv(add, wa) + conv(mx, wm)
                ob = opool.tile([D, H, W], F32, tag=f"outb{half}")
                out_img = out_pr[half * D:half * D + D]
                if variant in ('nofuse', 'novec'):
                    nc.vector.tensor_copy(ob[:], ap[:, 0:H, 0:W])
                    nc.sync.dma_start(out_img, ob[:])
                else:
                    idx = 0
                    for src, fbase in ((ap, 0), (mp, 9)):
                        for dh in range(3):
                            for dw in range(3):
                                iv = src[:, dh:dh + H, dw:dw + W]
                                fo = fbase + dh * 3 + dw
                                w_ap = fw_t[:, fo:fo + 1]
                                if idx == 0:
                                    nc.vector.tensor_scalar_mul(ob[:], iv, w_ap)
                                else:
                                    nc.vector.scalar_tensor_tensor(
                                        ob[:], iv, w_ap, ob[:], ALU.mult, ALU.add)
                                idx += 1
                    nc.sync.dma_start(out_img, ob[:])

    return nc


def _host_prep(inputs):
    """Rearrange weights / fold BN on the host (numpy only)."""
    w = [np.asarray(inputs[n], np.float32) for n in ("w1", "w2", "w3")]
    wa = np.zeros((128, N_OFF_A * D), np.float16)
    idx = 0
    for wi, (k, p) in zip(w, CONVS):
        for kh in range(k):
            for kw in range(k):
                wa[:, idx * D:(idx + 1) * D] = wi[:, :128, kh, kw].T
                idx += 1
    wb = np.zeros((112, N_COL_B * D), np.float16)
    col = 0
    for wi, (k, p) in zip(w, CONVS):
        for kh in range(k):
            for g in range(2 * p + 1):
                kw = ORDER[g] - 3 + p
                wb[g * 16:(g + 1) * 16, col * D:(col + 1) * D] = wi[:, 128:L, kh, kw].T
            col += 1

    sc = np.zeros((D, 6), np.float32)
    for ci, pre in enumerate(("1", "2", "3")):
        g = np.asarray(inputs["g" + pre], np.float32)
        be = np.asarray(inputs["be" + pre], np.float32)
        m = np.asarray(inputs["m" + pre], np.float32)
        v = np.asarray(inputs["v" + pre], np.float32)
        bconv = np.asarray(inputs["b" + pre], np.float32)
        inv = g / np.sqrt(v + EPS)
        sc[:, 2 * ci] = inv
        sc[:, 2 * ci + 1] = bconv * inv + be - m * inv

    wf = np.asarray(inputs["wf"], np.float32)
    wfa = wf[0, 0] + wf[0, 1] / 3.0    # [3,3]
    wfm = wf[0, 2]
    fw = np.zeros((D, 18), np.float32)
    fw[:, 0:9] = wfa.reshape(1, 9)
    fw[:, 9:18] = wfm.reshape(1, 9)
    return wa, wb, fw, sc


def _pad_images(x):
    """Host-side padding/pre-shifting of the full input batch."""
    nb = x.shape[0]
    xap = np.zeros((nb, 128, HP, WP), np.float16)
    xap[:, :, PAD:PAD + H, PAD:PAD + W] = x[:, :128]
    xbp = np.zeros((nb, 112, HP, W), np.float16)
    for g in range(7):
        s = ORDER[g] - 3
        c0, c1 = max(0, -s), W - max(0, s)
        xbp[:, g * 16:(g + 1) * 16, PAD:PAD + H, c0:c1] = x[:, 128:L, :, c0 + s:c1 + s]
    return xap, xbp


def _run(inputs, iters=1, variant='full'):
    key = f"nc{iters}-{variant}"
    if key not in _cache:
        _cache[key] = _build_program(iters, variant)
    nc = _cache[key]

    x = np.ascontiguousarray(np.asarray(inputs["x"], np.float32))
    wa, wb, fw, sc = _host_prep(inputs)
    xap, xbp = _pad_images(x)
    xap = xap.reshape(N_CORES, B_LOC * 128, HP, WP)
    xbp = xbp.reshape(N_CORES, B_LOC * 112, HP, W)

    core_ids = list(range(N_CORES))
    in_maps = []
    for c in core_ids:
        in_maps.append({
            "xap": xap[c],
            "xbp": xbp[c],
            "wa": wa, "wb": wb, "fw": fw, "sc": sc,
        })
    res = run_bass_kernel_spmd(nc, in_maps, core_ids)
    outs = [np.asarray(res.results[i]["out"]).reshape(B_LOC, D, H, W)
            for i in range(N_CORES)]
    return np.concatenate(outs, axis=0)


def kernel(**inputs):
    return _run(inputs, iters=1)
